# revision 1
# baseline (speedup 1.0000x reference)
"""Binarized 3x3 conv (BConv2d) on 8 TRN2 NeuronCores.

Problem: x (32, 32, 256, 256) f32, weight (32, 32, 3, 3) f32.
  out = conv2d(x, sign(weight), padding='same') / sqrt(32*9)

Strategy:
  - Data-parallel over batch: core i gets images 4i..4i+3 (no collectives).
  - Per core, pack 4 images x 32 input channels onto the 128 SBUF
    partitions.  Each 3x3 tap becomes ONE K=128, M=128 matmul with a
    block-diagonal (per-image) binarized weight matrix; the 9 taps
    accumulate into PSUM and differ only in the rhs address offset into a
    zero-padded fp16 copy of the input (258x258 per partition).
  - fp16 inputs (weights are exactly +-1 in fp16), fp32 PSUM accumulate,
    fp16 output (host upcasts to f32; ~3e-4 rel err vs 2e-2 gate).
  - x is cast to fp16 host-side so DMA writes straight into the padded
    image (no staging tile, no cast pass, half the input HBM traffic).
  - Pipeline: DMA fp16 rows -> padded image, 9x4 matmuls per 8-row
    super-chunk into 4 PSUM banks (tapered 4,2,2 at the end), VectorE
    scaled drain to fp16, DMA out.  Zero-weight warm-up matmuls keep the
    PE clock gate hot while the first input rows are in flight.
"""

import numpy as np
import ml_dtypes

import concourse.mybir as mybir
import concourse.tile as tile
from concourse import bacc
from concourse import bass_utils

N_CORES = 8
N_IMG = 4          # images per core
C_IN = 32
C_OUT = 32
K = 3
H = 256
W = 256
DIV = float(np.sqrt(C_IN * K * K))


def build_conv_kernel(
    nimg=N_IMG,
    cin=C_IN,
    cout=C_OUT,
    h=H,
    w=W,
    chunk_rows=16,  # input DMA granularity (rows; fp16 -> 1 MB per DMA)
    bank_rows=2,    # output rows per PSUM bank matmul (bank_rows*w <= 512)
    banks_per_sc=4, # PSUM banks per super-chunk
    div=DIV,
    repeats=1,      # execute the whole body N times (for delta-timing)
    warmup_mms=8,   # zero-weight matmuls to warm the PE during input wait
):
    """Build the per-core Bass graph.  Returns nc (compiled Bacc)."""
    P = nimg * cin
    assert P <= 128
    M = nimg * cout
    assert M <= 128
    assert bank_rows * w <= 512
    sc_rows = bank_rows * banks_per_sc
    assert h % chunk_rows == 0 and h % sc_rows == 0
    hp, wp = h + 2, w + 2
    n_taps = 9

    nc = bacc.Bacc(
        "TRN2", target_bir_lowering=False, debug=False, num_devices=N_CORES
    )
    # x arrives pre-converted to fp16 (host-side marshalling): halves input
    # HBM traffic and lets DMA write straight into the padded image with no
    # staging tile or ScalarE cast pass.
    x_dram = nc.dram_tensor("x", [P, h, w], mybir.dt.float16, kind="ExternalInput")
    w_dram = nc.dram_tensor(
        "w9", [P, n_taps, M], mybir.dt.float16, kind="ExternalInput"
    )
    out_dram = nc.dram_tensor(
        "out", [M, h, w], mybir.dt.float16, kind="ExternalOutput"
    )

    with tile.TileContext(nc) as tc:
        with (
            tc.tile_pool(name="persist", bufs=1) as perpool,
            tc.tile_pool(name="ostage", bufs=3) as opool,
            tc.tile_pool(name="psum", bufs=2 * banks_per_sc, space="PSUM") as ppool,
        ):
            xpad = perpool.tile([P, hp, wp], mybir.dt.float16, name="xpad")
            wsb = perpool.tile([P, n_taps, M], mybir.dt.float16, name="wsb")
            wz = perpool.tile([P, M], mybir.dt.float16, name="wz")
            nc.vector.memset(wz[:], 0.0)

            # tap-0 weights first so the first LDWEIGHTS unblocks early
            nc.sync.dma_start(out=wsb[:, 0, :], in_=w_dram[:, 0, :])
            nc.sync.dma_start(out=wsb[:, 1:, :], in_=w_dram[:, 1:, :])
            # zero the padding ring (top/bottom rows, left/right columns)
            nc.vector.memset(xpad[:, 0, :], 0.0)
            nc.vector.memset(xpad[:, hp - 1, :], 0.0)
            nc.vector.memset(xpad[:, :, 0], 0.0)
            nc.vector.memset(xpad[:, :, wp - 1], 0.0)

            def emit_input_rows(r0, nrows):
                nc.sync.dma_start(
                    out=xpad[:, r0 + 1 : r0 + nrows + 1, 1 : w + 1],
                    in_=x_dram[:, r0 : r0 + nrows, :],
                )

            def emit_body():
                # Interleave input chunks with compute super-chunks so DMA
                # lane semaphore windows complete progressively (an
                # up-front input burst couples early sem resets to the
                # last input DMA and stalls the whole pipeline mid-kernel).
                next_row = [0]

                def load_until(row_needed):
                    while next_row[0] < min(row_needed, h):
                        r0 = next_row[0]
                        # small first pieces so the first matmuls start early
                        nrows = 2 if r0 < sc_rows else chunk_rows
                        nrows = min(nrows, h - r0)
                        emit_input_rows(r0, nrows)
                        next_row[0] += nrows

                # compute pipeline: super-chunks of output rows, one
                # PSUM-bank tile per bank_rows strip (own accum group).
                # The last super-chunk tapers (4,2,2 rows) so the final
                # drain->store chain after the last matmul is short.
                plan = []
                r = 0
                while r < h:
                    if h - r > sc_rows:
                        rows = sc_rows
                    elif h - r == sc_rows and sc_rows >= 8:
                        plan += [(r, sc_rows // 2), (r + sc_rows // 2,
                                                     sc_rows // 4)]
                        r += 3 * sc_rows // 4
                        rows = h - r
                    else:
                        rows = h - r
                    plan.append((r, rows))
                    r += rows

                # PE warm-up while the first input chunks are in flight:
                # zero-weight matmuls on the (memset) pad row keep the PE
                # busy so the HAM clock gate reaches 2.4 GHz before real
                # work.  Reads/writes only zeros; scratch bank is unused.
                if warmup_mms:
                    wpt = ppool.tile(
                        [M, bank_rows, w], mybir.dt.float32,
                        name="wpt", tag="pt",
                    )
                    for _ in range(warmup_mms):
                        nc.tensor.matmul(
                            wpt[:, 0, 0:w], wz[:], xpad[:, 0, 0:w],
                            start=True, stop=True,
                        )

                for si, (h0, rows) in enumerate(plan):
                    load_until(min(h0 + rows + sc_rows + 1, h))
                    banks = rows // bank_rows
                    pts = [
                        ppool.tile(
                            [M, bank_rows, w], mybir.dt.float32,
                            name="pt", tag="pt",
                        )
                        for _ in range(banks)
                    ]
                    # first SC: bank-outer so bank 0's accumulation (which
                    # needs only the first 3 input rows) completes first
                    if si == 0:
                        order = [(t, b) for b in range(banks)
                                 for t in range(n_taps)]
                    else:
                        order = [(t, b) for t in range(n_taps)
                                 for b in range(banks)]
                    for t, b in order:
                        dy, dx = t // 3, t % 3
                        hb = h0 + b * bank_rows
                        nc.tensor.matmul(
                            pts[b][:],
                            wsb[:, t, :],
                            xpad[:, hb + dy : hb + dy + bank_rows, dx : dx + w],
                            start=(t == 0),
                            stop=(t == n_taps - 1),
                        )
                    ot = opool.tile(
                        [M, rows, w], mybir.dt.float16, name="ot", tag="ot",
                        padded_shape=[M, sc_rows, w],
                    )
                    for b in range(banks):
                        nc.vector.tensor_scalar_mul(
                            ot[:, b * bank_rows : (b + 1) * bank_rows, :],
                            pts[b][:],
                            1.0 / div,
                        )
                    nc.sync.dma_start(
                        out=out_dram[:, h0 : h0 + rows, :], in_=ot[:]
                    )
                load_until(h)

            for _rep in range(repeats):
                emit_body()

    nc.compile()
    return nc


def make_weight_tensor(weight, nimg=N_IMG, cin=C_IN, cout=C_OUT):
    """Binarize + block-diagonalize: [cout,cin,3,3] f32 -> [nimg*cin, 9, nimg*cout] bf16."""
    n_taps = weight.shape[2] * weight.shape[3]
    wbin = np.where(weight > 0, 1.0, -1.0).astype(np.float32)
    # [co, ci, kh, kw] -> [ci, t, co]
    wt = wbin.reshape(cout, cin, n_taps).transpose(1, 2, 0)
    w9 = np.zeros((nimg * cin, n_taps, nimg * cout), dtype=np.float16)
    for i in range(nimg):
        w9[i * cin : (i + 1) * cin, :, i * cout : (i + 1) * cout] = wt
    return w9


def kernel(x, weight, trace=False, repeats=1, _nc_cache={}):
    """Full-input entry point: x (32,32,256,256) f32, weight (32,32,3,3) f32."""
    x = np.asarray(x, dtype=np.float32)
    # host-side fp16 marshalling (fp16 >= bf16 precision at identical speed)
    x = np.ascontiguousarray(x.astype(np.float16))
    weight = np.asarray(weight, dtype=np.float32)
    n_batch = x.shape[0]
    per_core = n_batch // N_CORES

    if repeats not in _nc_cache:
        _nc_cache[repeats] = build_conv_kernel(repeats=repeats)
    nc = _nc_cache[repeats]

    w9 = make_weight_tensor(weight)
    P = N_IMG * C_IN
    in_maps = [
        {
            "x": x[i * per_core : (i + 1) * per_core].reshape(P, H, W),
            "w9": w9,
        }
        for i in range(N_CORES)
    ]
    try:
        res = bass_utils.run_bass_kernel_spmd(
            nc, in_maps, core_ids=list(range(N_CORES)), trace=trace
        )
    except ModuleNotFoundError:
        # axon NTFF profiling hook unavailable in this environment
        res = bass_utils.run_bass_kernel_spmd(
            nc, in_maps, core_ids=list(range(N_CORES)), trace=False
        )
    out = np.concatenate(
        [r["out"].astype(np.float32).reshape(per_core, C_OUT, H, W)
         for r in res.results],
        axis=0,
    )
    if trace:
        kernel.last_results = res
    return out



# revision 31
# speedup vs baseline: 2.0341x; 2.0341x over previous
"""Binarized 3x3 conv (BConv2d) on 8 TRN2 NeuronCores — fp8 DoubleRow version.

Problem: x (32, 32, 256, 256) f32, weight (32, 32, 3, 3) f32.
  out = conv2d(x, sign(weight), padding='same') / sqrt(32*9)

Strategy:
  - Data-parallel over batch: core i gets images 4i..4i+3 (no collectives).
  - Block-diagonal image packing: K = M = 128 = 4 images x 32 channels.
  - fp8 e4m3 DoubleRow matmuls: one instruction computes
    W_A.T @ X_A + W_B.T @ X_B at 0.5 cycles/column (4x bf16 rate).
    The A/B planes are expressed as a [K, 2, N] rhs access pattern whose
    middle dim strides between two tap windows of the same SBUF image
    (hardware requires that stride to be EVEN; odd strides crash).
  - Dual-plane numerics: host ships a = e4m3(x) and b = e4m3(x - a).
    7 of the 9 taps also accumulate the b (residual) plane, recovering
    most of the e4m3 quantization error: rel err ~1.3e-2 vs 2e-2 gate
    (full e4m3 alone is 2.7e-2).  16 tap-planes = 8 DoubleRow per output
    row vs 9 fp16 matmuls per row in the fp16 design (3.6x fewer PE
    cycles).
  - Input layout is W-unpadded (rows contiguous) so fp8 chunks DMA at
    full descriptor speed; column-edge outputs (c=0, c=255) read
    wrapped garbage and are recomputed by small per-superchunk edge
    matmuls (N = rows), then overwritten during the drain.
  - fp32 PSUM accumulate, drains split across DVE and ACT engines with
    the 1/sqrt(288) scale fused, fp16 output DMA.
"""

import numpy as np
import ml_dtypes

import concourse.mybir as mybir
import concourse.tile as tile
from concourse import bacc
from concourse import bass_utils
from concourse.ap import AP

N_CORES = 8
N_IMG = 4
C_IN = 32
C_OUT = 32
K = 3
H = 256
W = 256
DIV = float(np.sqrt(C_IN * K * K))

P = N_IMG * C_IN      # 128 partitions (contraction)
M = N_IMG * C_OUT     # 128 psum partitions (outputs)
HP = H + 2            # zero row above and below
RS = 2 * W            # row stride: a-row and b-row interleaved per image row
                      # (keeps every DoubleRow pair delta < 2^16: the ISA
                      # step_elem field is 16-bit, a flat-plane delta of
                      # 66048 fails codegen)
GUARD = 8             # guard bytes (tap windows spill 1 elem at the edges)
XF = 2 * GUARD + HP * RS
A0 = GUARD            # a sub-row base offset within a row slot
B0 = GUARD + W        # b sub-row base offset within a row slot

# ---- bulk tap-plane pairing (one DoubleRow instruction per entry) ----
# ('a'|'b'|None, dy, dx): None = zero-weight plane (reads in-bounds bytes).
# Constraint: within a pair, the flat-offset delta must be even, i.e. the
# dx values must have equal parity.  B-set (residual-corrected taps) =
# {(0,1),(1,1),(2,1),(0,0),(1,0),(2,0),(1,2)}; uncorrected: (0,2),(2,2).
BULK_PAIRS = [
    (("a", 0, 0), ("a", 0, 2)),
    (("a", 1, 0), ("a", 1, 2)),
    (("a", 2, 0), ("a", 2, 2)),
    (("a", 0, 1), ("a", 1, 1)),
    (("a", 2, 1), ("b", 2, 1)),
    (("b", 0, 1), ("b", 1, 1)),
    (("b", 0, 0), ("b", 1, 0)),
    (("b", 1, 2), ("b", 2, 0)),
]
# ---- edge fixup pairs (a-plane only; error contribution ~2/256 pixels) --
# col 0 rebuilt from taps dx in {1,2} (input cols 0,1); col 255 from taps
# dx in {0,1} (input cols 254,255).  EDGE_COL maps slot -> (psum row, cin0)
EDGE_PAIRS = [
    # column 0 (group 0): window offset = dy*W + (dx-1) + 0
    (("a", 0, 1), ("a", 1, 1), 0, 0),
    (("a", 2, 1), (None, 2, 1), 0, 0),
    (("a", 0, 2), ("a", 1, 2), 0, 0),
    (("a", 2, 2), (None, 2, 2), 0, 0),
    # column 255 (group 1): window offset = dy*W + (dx-1) + 255
    (("a", 0, 0), ("a", 1, 0), 1, 255),
    (("a", 2, 0), (None, 2, 0), 1, 255),
    (("a", 0, 1), ("a", 1, 1), 1, 255),
    (("a", 2, 1), (None, 2, 1), 1, 255),
]
N_BULK = len(BULK_PAIRS)            # 8
N_EDGE = len(EDGE_PAIRS)            # 8
N_SLOTS = N_BULK + N_EDGE + 1       # +1 all-zero warmup slot
WARM_SLOT = N_SLOTS - 1


def _plane_off(plane, dy, dx):
    """Flat offset of tap window start relative to output row r's base
    (r*RS), for output column 0."""
    base = A0 if plane == "a" else B0
    return base + dy * RS + (dx - 1)


def build_conv_kernel(
    sc_rows=16,        # output rows per superchunk (drain+DMA granularity)
    chunk_rows=4,      # input DMA chunk rows (both planes, contiguous)
    first_chunk=4,     # extra rows in the upfront load
    warmup_mms=24,     # zero-weight DoubleRow matmuls to ramp the PE clock
    psum_bufs=6,       # bulk PSUM rotation depth
    obufs=4,           # output staging buffers
    in_queue="act",    # engine queue for input DMAs ('act' | 'sp')
    wave=4,            # pairs per staggered wave (psum banks held)
    drain_split=True,  # alternate drains DVE/ACT (False: all DVE)
    repeats=1,
):
    assert H % sc_rows == 0 and sc_rows % 2 == 0
    n_sc = H // sc_rows

    nc = bacc.Bacc(
        "TRN2", target_bir_lowering=False, debug=False, num_devices=N_CORES
    )
    x_dram = nc.dram_tensor(
        "xab", [P, H, 2, W], mybir.dt.float8e4, kind="ExternalInput"
    )
    w_dram = nc.dram_tensor(
        "wall", [P, N_SLOTS, 2, M], mybir.dt.float8e4, kind="ExternalInput"
    )
    out_dram = nc.dram_tensor(
        "out", [M, H, W], mybir.dt.float16, kind="ExternalOutput"
    )

    with tile.TileContext(nc) as tc:
        with (
            tc.tile_pool(name="persist", bufs=1) as perpool,
            tc.tile_pool(name="ostage", bufs=obufs) as opool,
            tc.tile_pool(name="psum", bufs=psum_bufs, space="PSUM") as ppool,
            tc.tile_pool(name="epsum", bufs=2, space="PSUM") as epool,
        ):
            xab = perpool.tile([P, XF], mybir.dt.float8e4, name="xab")
            wsb = perpool.tile(
                [P, N_SLOTS, 2, M], mybir.dt.float8e4, name="wsb"
            )
            nc.sync.dma_start(out=wsb[:], in_=w_dram[:])

            # zero the guards and the top/bottom pad row slots (both the
            # a and b sub-rows of padded rows 0 and 257)
            nc.vector.memset(xab[:, 0:A0], 0.0)
            nc.vector.memset(xab[:, A0 : A0 + RS], 0.0)
            nc.vector.memset(xab[:, A0 + (HP - 1) * RS : XF], 0.0)

            xt = xab[:].tensor

            def rhs_bulk(r, pair):
                (p1, dy1, dx1), (p2, dy2, dx2) = pair
                o1 = r * RS + _plane_off(p1, dy1, dx1)
                o2 = r * RS + _plane_off(p2, dy2, dx2)
                assert (o2 - o1) % 2 == 0 and o1 >= 0 and 0 < o2 - o1 < 65536
                return AP(xt, o1, [[XF, P], [o2 - o1, 2], [1, W]])

            def rhs_edge(r0, rows, entry):
                (p1, dy1, dx1), (p2, dy2, dx2), _grp, c = entry
                o1 = r0 * RS + _plane_off(p1, dy1, dx1) + c
                if p2 is None:
                    o2 = o1 + 2   # zero-weight plane: any even in-bounds delta
                else:
                    o2 = r0 * RS + _plane_off(p2, dy2, dx2) + c
                assert (o2 - o1) % 2 == 0 and o1 >= 0 and 0 < o2 - o1 < 65536
                return AP(xt, o1, [[XF, P], [o2 - o1, 2], [RS, rows]])

            def emit_input_rows(r0, nrows):
                # the row-interleaved [P, H, 2, W] dram layout makes each
                # chunk one fully contiguous transfer (both planes)
                eng = {"act": nc.scalar, "sp": nc.sync,
                       "pool": nc.gpsimd}[in_queue]
                dst = AP(
                    xt, A0 + (r0 + 1) * RS, [[XF, P], [1, nrows * RS]]
                )
                eng.dma_start(
                    out=dst,
                    in_=x_dram[:, r0 : r0 + nrows, :, :],
                )

            def emit_body():
                next_row = [0]

                def load_until(row_needed):
                    while next_row[0] < min(row_needed, H):
                        r0 = next_row[0]
                        nrows = min(chunk_rows, H - r0)
                        emit_input_rows(r0, nrows)
                        next_row[0] += nrows

                # PE p-state warm-up on the (memset) guard bytes.  Emitted
                # BEFORE the upfront loads: any xab read emitted after a DMA
                # inherits a wait on it (latest-emitted coalescing), which
                # would make the warmups wait out the whole upfront burst.
                if warmup_mms:
                    wpt = ppool.tile(
                        [M, 2, W], mybir.dt.float32, name="pt", tag="pt"
                    )
                    # memset zero-weight tile: warmups depend on no DMA, so
                    # the p-state ramp starts while weights are in flight
                    wz = perpool.tile([P, 2, M], mybir.dt.float8e4, name="wz")
                    nc.vector.memset(wz[:], 0.0)
                    # read only the (memset) a-plane top pad row: base A0,
                    # plane delta 2, N=254 keeps every byte inside the pad
                    wrhs = AP(xt, A0, [[XF, P], [2, 2], [1, W - 2]])
                    for _ in range(warmup_mms):
                        nc.tensor.matmul(
                            wpt[:, 0, 0 : W - 2],
                            wz[:],
                            wrhs,
                            start=True,
                            stop=True,
                            perf_mode=mybir.MatmulPerfMode.DoubleRow,
                        )

                # upfront: superchunk 0, its halo, and one chunk of slack
                load_until(2 * sc_rows + first_chunk)

                for sc in range(n_sc):
                    h0 = sc * sc_rows
                    ot = opool.tile(
                        [M, sc_rows, W], mybir.dt.float16, name="ot", tag="ot"
                    )
                    n_pairs = sc_rows // 2
                    for w0 in range(0, n_pairs, wave):
                        pts = [
                            ppool.tile(
                                [M, 2, W], mybir.dt.float32,
                                name="pt", tag="pt",
                            )
                            for _ in range(min(wave, n_pairs - w0))
                        ]
                        # phase 1: a-plane slots for the whole wave; phase 2:
                        # b-plane slots.  This defers the wave's first
                        # b-chunk semaphore wait by ~2 us of PE work, past
                        # the prefetched chunk's landing time.
                        for s, pair in enumerate(BULK_PAIRS):
                            for jj, pt in enumerate(pts):
                                r0 = h0 + 2 * (w0 + jj)
                                for i in range(2):
                                    nc.tensor.matmul(
                                        pt[:, i, :],
                                        wsb[:, s, :, :],
                                        rhs_bulk(r0 + i, pair),
                                        start=(s == 0),
                                        stop=(s == N_BULK - 1),
                                        perf_mode=mybir.MatmulPerfMode.DoubleRow,
                                    )
                        for jj, pt in enumerate(pts):
                            j = w0 + jj
                            dst = ot[:, 2 * j : 2 * j + 2, :]
                            if j % 2 == 0 or not drain_split:
                                nc.vector.tensor_scalar_mul(
                                    dst, pt[:], 1.0 / DIV
                                )
                            else:
                                nc.scalar.activation(
                                    dst,
                                    pt[:],
                                    mybir.ActivationFunctionType.Copy,
                                    scale=1.0 / DIV,
                                )
                        if w0 == 0:
                            # JIT prefetch emitted mid-sc
                            load_until(h0 + 3 * sc_rows + first_chunk)

                    # edge fixup: recompute output columns 0 and 255
                    et = epool.tile(
                        [M, 2, sc_rows], mybir.dt.float32, name="et", tag="et"
                    )
                    ecount = [0, 0]
                    for e, entry in enumerate(EDGE_PAIRS):
                        grp = entry[2]
                        s = N_BULK + e
                        nc.tensor.matmul(
                            et[:, grp, :],
                            wsb[:, s, :, :],
                            rhs_edge(h0, sc_rows, entry),
                            start=(ecount[grp] == 0),
                            stop=(ecount[grp] == 3),
                            perf_mode=mybir.MatmulPerfMode.DoubleRow,
                        )
                        ecount[grp] += 1
                    for grp, c in ((0, 0), (1, W - 1)):
                        nc.vector.tensor_scalar_mul(
                            ot[:, :, c : c + 1],
                            et[:, grp, :].unsqueeze(2),
                            1.0 / DIV,
                        )
                    nc.sync.dma_start(
                        out=out_dram[:, h0 : h0 + sc_rows, :], in_=ot[:]
                    )
                load_until(H)

            for _rep in range(repeats):
                emit_body()

    nc.compile()
    return nc


def make_weight_tensor(weight):
    """Binarize + block-diagonalize into the paired slot layout."""
    wbin = np.where(weight > 0, 1.0, -1.0).astype(np.float32)
    # [co, ci, kh, kw] -> per-tap [ci, co]
    wt = wbin.reshape(C_OUT, C_IN, K * K).transpose(1, 2, 0)
    w128 = np.zeros((K * K, P, M), dtype=np.float32)
    for i in range(N_IMG):
        w128[:, i * C_IN : (i + 1) * C_IN, i * C_OUT : (i + 1) * C_OUT] = (
            wt.transpose(1, 0, 2)
        )
    wall = np.zeros((P, N_SLOTS, 2, M), dtype=np.float32)
    for s, pair in enumerate(BULK_PAIRS):
        for pl, (pk, dy, dx) in enumerate(pair):
            wall[:, s, pl, :] = w128[dy * K + dx]
    for e, entry in enumerate(EDGE_PAIRS):
        for pl, (pk, dy, dx) in enumerate(entry[:2]):
            if pk is not None:
                wall[:, N_BULK + e, pl, :] = w128[dy * K + dx]
    return wall.astype(ml_dtypes.float8_e4m3)


def kernel(x, weight, trace=False, repeats=1, _nc_cache={}):
    """Full-input entry point: x (32,32,256,256) f32, weight (32,32,3,3) f32."""
    x = np.asarray(x, dtype=np.float32)
    weight = np.asarray(weight, dtype=np.float32)
    n_batch = x.shape[0]
    per_core = n_batch // N_CORES

    if repeats not in _nc_cache:
        _nc_cache[repeats] = build_conv_kernel(repeats=repeats)
    nc = _nc_cache[repeats]

    wall = make_weight_tensor(weight)
    # dual-plane e4m3 host marshalling: a = fp8(x), b = fp8(x - a),
    # row-interleaved: [n, cin, h, 2, w]
    a = x.astype(ml_dtypes.float8_e4m3)
    b = (x - a.astype(np.float32)).astype(ml_dtypes.float8_e4m3)
    xab = np.stack([a, b], axis=3)

    in_maps = [
        {
            "xab": np.ascontiguousarray(
                xab[i * per_core : (i + 1) * per_core].reshape(P, H, 2, W)
            ),
            "wall": wall,
        }
        for i in range(N_CORES)
    ]
    try:
        res = bass_utils.run_bass_kernel_spmd(
            nc, in_maps, core_ids=list(range(N_CORES)), trace=trace
        )
    except ModuleNotFoundError:
        res = bass_utils.run_bass_kernel_spmd(
            nc, in_maps, core_ids=list(range(N_CORES)), trace=False
        )
    out = np.concatenate(
        [r["out"].astype(np.float32).reshape(per_core, C_OUT, H, W)
         for r in res.results],
        axis=0,
    )
    if trace:
        kernel.last_results = res
    return out


# revision 32
# speedup vs baseline: 2.2726x; 1.1173x over previous
"""Binarized 3x3 conv (BConv2d) on 8 TRN2 NeuronCores — fp8 DoubleRow version.

Problem: x (32, 32, 256, 256) f32, weight (32, 32, 3, 3) f32.
  out = conv2d(x, sign(weight), padding='same') / sqrt(32*9)

Strategy:
  - Data-parallel over batch: core i gets images 4i..4i+3 (no collectives).
  - Block-diagonal image packing: K = M = 128 = 4 images x 32 channels.
  - fp8 e4m3 DoubleRow matmuls: one instruction computes
    W_A.T @ X_A + W_B.T @ X_B at 0.5 cycles/column (4x bf16 rate).
    The A/B planes are expressed as a [K, 2, N] rhs access pattern whose
    middle dim strides between two tap windows of the same SBUF image
    (hardware requires that stride to be EVEN; odd strides crash).
  - Dual-plane numerics: host ships a = e4m3(x) and b = e4m3(x - a).
    7 of the 9 taps also accumulate the b (residual) plane, recovering
    most of the e4m3 quantization error: rel err ~1.3e-2 vs 2e-2 gate
    (full e4m3 alone is 2.7e-2).  16 tap-planes = 8 DoubleRow per output
    row vs 9 fp16 matmuls per row in the fp16 design (3.6x fewer PE
    cycles).
  - Input layout is W-unpadded (rows contiguous) so fp8 chunks DMA at
    full descriptor speed; column-edge outputs (c=0, c=255) read
    wrapped garbage and are recomputed by small per-superchunk edge
    matmuls (N = rows), then overwritten during the drain.
  - fp32 PSUM accumulate, drains split across DVE and ACT engines with
    the 1/sqrt(288) scale fused, fp16 output DMA.
"""

import numpy as np
import ml_dtypes

import concourse.mybir as mybir
import concourse.tile as tile
from concourse import bacc
from concourse import bass_utils
from concourse.ap import AP

N_CORES = 8
N_IMG = 4
C_IN = 32
C_OUT = 32
K = 3
H = 256
W = 256
DIV = float(np.sqrt(C_IN * K * K))

P = N_IMG * C_IN      # 128 partitions (contraction)
M = N_IMG * C_OUT     # 128 psum partitions (outputs)
HP = H + 2            # zero row above and below
RS = 2 * W            # row stride: a-row and b-row interleaved per image row
                      # (keeps every DoubleRow pair delta < 2^16: the ISA
                      # step_elem field is 16-bit, a flat-plane delta of
                      # 66048 fails codegen)
GUARD = 8             # guard bytes (tap windows spill 1 elem at the edges)
XF = 2 * GUARD + HP * RS
A0 = GUARD            # a sub-row base offset within a row slot
B0 = GUARD + W        # b sub-row base offset within a row slot

# ---- bulk tap-plane pairing (one DoubleRow instruction per entry) ----
# ('a'|'b'|None, dy, dx): None = zero-weight plane (reads in-bounds bytes).
# Constraint: within a pair, the flat-offset delta must be even, i.e. the
# dx values must have equal parity.  B-set (residual-corrected taps) =
# {(1,1),(0,0),(1,0),(2,0),(1,2)}; uncorrected: (0,1),(2,1),(0,2),(2,2)
# (rel err ~1.77e-2 vs the 2e-2 gate; 7 instructions/row).
BULK_PAIRS = [
    (("a", 0, 0), ("a", 0, 2)),
    (("a", 1, 0), ("a", 1, 2)),
    (("a", 2, 0), ("a", 2, 2)),
    (("a", 0, 1), ("a", 1, 1)),
    (("b", 1, 1), ("a", 2, 1)),
    (("b", 0, 0), ("b", 1, 0)),
    (("b", 1, 2), ("b", 2, 0)),
]
# ---- edge fixup pairs (a-plane only; error contribution ~2/256 pixels) --
# col 0 rebuilt from taps dx in {1,2} (input cols 0,1); col 255 from taps
# dx in {0,1} (input cols 254,255).  EDGE_COL maps slot -> (psum row, cin0)
EDGE_PAIRS = [
    # column 0 (group 0): window offset = dy*W + (dx-1) + 0
    (("a", 0, 1), ("a", 1, 1), 0, 0),
    (("a", 2, 1), (None, 2, 1), 0, 0),
    (("a", 0, 2), ("a", 1, 2), 0, 0),
    (("a", 2, 2), (None, 2, 2), 0, 0),
    # column 255 (group 1): window offset = dy*W + (dx-1) + 255
    (("a", 0, 0), ("a", 1, 0), 1, 255),
    (("a", 2, 0), (None, 2, 0), 1, 255),
    (("a", 0, 1), ("a", 1, 1), 1, 255),
    (("a", 2, 1), (None, 2, 1), 1, 255),
]
N_BULK = len(BULK_PAIRS)            # 8
N_EDGE = len(EDGE_PAIRS)            # 8
N_SLOTS = N_BULK + N_EDGE + 1       # +1 all-zero warmup slot
WARM_SLOT = N_SLOTS - 1


def _plane_off(plane, dy, dx):
    """Flat offset of tap window start relative to output row r's base
    (r*RS), for output column 0."""
    base = A0 if plane == "a" else B0
    return base + dy * RS + (dx - 1)


def build_conv_kernel(
    sc_rows=16,        # output rows per superchunk (drain+DMA granularity)
    chunk_rows=4,      # input DMA chunk rows (both planes, contiguous)
    first_chunk=4,     # extra rows in the upfront load
    warmup_mms=24,     # zero-weight DoubleRow matmuls to ramp the PE clock
    psum_bufs=6,       # bulk PSUM rotation depth
    obufs=4,           # output staging buffers
    in_queue="act",    # engine queue for input DMAs ('act' | 'sp')
    wave=4,            # pairs per staggered wave (psum banks held)
    drain_split=True,  # alternate drains DVE/ACT (False: all DVE)
    repeats=1,
):
    assert H % sc_rows == 0 and sc_rows % 2 == 0
    n_sc = H // sc_rows

    nc = bacc.Bacc(
        "TRN2", target_bir_lowering=False, debug=False, num_devices=N_CORES
    )
    x_dram = nc.dram_tensor(
        "xab", [P, H, 2, W], mybir.dt.float8e4, kind="ExternalInput"
    )
    w_dram = nc.dram_tensor(
        "wall", [P, N_SLOTS, 2, M], mybir.dt.float8e4, kind="ExternalInput"
    )
    out_dram = nc.dram_tensor(
        "out", [M, H, W], mybir.dt.float16, kind="ExternalOutput"
    )

    with tile.TileContext(nc) as tc:
        with (
            tc.tile_pool(name="persist", bufs=1) as perpool,
            tc.tile_pool(name="ostage", bufs=obufs) as opool,
            tc.tile_pool(name="psum", bufs=psum_bufs, space="PSUM") as ppool,
            tc.tile_pool(name="epsum", bufs=2, space="PSUM") as epool,
        ):
            xab = perpool.tile([P, XF], mybir.dt.float8e4, name="xab")
            wsb = perpool.tile(
                [P, N_SLOTS, 2, M], mybir.dt.float8e4, name="wsb"
            )
            nc.sync.dma_start(out=wsb[:], in_=w_dram[:])

            # zero the guards and the top/bottom pad row slots (both the
            # a and b sub-rows of padded rows 0 and 257)
            nc.vector.memset(xab[:, 0:A0], 0.0)
            nc.vector.memset(xab[:, A0 : A0 + RS], 0.0)
            nc.vector.memset(xab[:, A0 + (HP - 1) * RS : XF], 0.0)

            xt = xab[:].tensor

            def rhs_bulk(r, pair):
                (p1, dy1, dx1), (p2, dy2, dx2) = pair
                o1 = r * RS + _plane_off(p1, dy1, dx1)
                o2 = r * RS + _plane_off(p2, dy2, dx2)
                assert (o2 - o1) % 2 == 0 and o1 >= 0 and 0 < o2 - o1 < 65536
                return AP(xt, o1, [[XF, P], [o2 - o1, 2], [1, W]])

            def rhs_edge(r0, rows, entry):
                (p1, dy1, dx1), (p2, dy2, dx2), _grp, c = entry
                o1 = r0 * RS + _plane_off(p1, dy1, dx1) + c
                if p2 is None:
                    o2 = o1 + 2   # zero-weight plane: any even in-bounds delta
                else:
                    o2 = r0 * RS + _plane_off(p2, dy2, dx2) + c
                assert (o2 - o1) % 2 == 0 and o1 >= 0 and 0 < o2 - o1 < 65536
                return AP(xt, o1, [[XF, P], [o2 - o1, 2], [RS, rows]])

            def emit_input_rows(r0, nrows):
                # the row-interleaved [P, H, 2, W] dram layout makes each
                # chunk one fully contiguous transfer (both planes)
                eng = {"act": nc.scalar, "sp": nc.sync,
                       "pool": nc.gpsimd}[in_queue]
                dst = AP(
                    xt, A0 + (r0 + 1) * RS, [[XF, P], [1, nrows * RS]]
                )
                eng.dma_start(
                    out=dst,
                    in_=x_dram[:, r0 : r0 + nrows, :, :],
                )

            def emit_body():
                next_row = [0]

                def load_until(row_needed):
                    while next_row[0] < min(row_needed, H):
                        r0 = next_row[0]
                        nrows = min(chunk_rows, H - r0)
                        emit_input_rows(r0, nrows)
                        next_row[0] += nrows

                # PE p-state warm-up on the (memset) guard bytes.  Emitted
                # BEFORE the upfront loads: any xab read emitted after a DMA
                # inherits a wait on it (latest-emitted coalescing), which
                # would make the warmups wait out the whole upfront burst.
                if warmup_mms:
                    wpt = ppool.tile(
                        [M, 2, W], mybir.dt.float32, name="pt", tag="pt"
                    )
                    # memset zero-weight tile: warmups depend on no DMA, so
                    # the p-state ramp starts while weights are in flight
                    wz = perpool.tile([P, 2, M], mybir.dt.float8e4, name="wz")
                    nc.vector.memset(wz[:], 0.0)
                    # read only the (memset) a-plane top pad row: base A0,
                    # plane delta 2, N=254 keeps every byte inside the pad
                    wrhs = AP(xt, A0, [[XF, P], [2, 2], [1, W - 2]])
                    for _ in range(warmup_mms):
                        nc.tensor.matmul(
                            wpt[:, 0, 0 : W - 2],
                            wz[:],
                            wrhs,
                            start=True,
                            stop=True,
                            perf_mode=mybir.MatmulPerfMode.DoubleRow,
                        )

                # upfront: superchunk 0, its halo, and one chunk of slack
                load_until(2 * sc_rows + first_chunk)

                for sc in range(n_sc):
                    h0 = sc * sc_rows
                    ot = opool.tile(
                        [M, sc_rows, W], mybir.dt.float16, name="ot", tag="ot"
                    )
                    n_pairs = sc_rows // 2
                    for w0 in range(0, n_pairs, wave):
                        pts = [
                            ppool.tile(
                                [M, 2, W], mybir.dt.float32,
                                name="pt", tag="pt",
                            )
                            for _ in range(min(wave, n_pairs - w0))
                        ]
                        # phase 1: a-plane slots for the whole wave; phase 2:
                        # b-plane slots.  This defers the wave's first
                        # b-chunk semaphore wait by ~2 us of PE work, past
                        # the prefetched chunk's landing time.
                        for s, pair in enumerate(BULK_PAIRS):
                            for jj, pt in enumerate(pts):
                                r0 = h0 + 2 * (w0 + jj)
                                for i in range(2):
                                    nc.tensor.matmul(
                                        pt[:, i, :],
                                        wsb[:, s, :, :],
                                        rhs_bulk(r0 + i, pair),
                                        start=(s == 0),
                                        stop=(s == N_BULK - 1),
                                        perf_mode=mybir.MatmulPerfMode.DoubleRow,
                                    )
                        for jj, pt in enumerate(pts):
                            j = w0 + jj
                            dst = ot[:, 2 * j : 2 * j + 2, :]
                            if j % 2 == 0 or not drain_split:
                                nc.vector.tensor_scalar_mul(
                                    dst, pt[:], 1.0 / DIV
                                )
                            else:
                                nc.scalar.activation(
                                    dst,
                                    pt[:],
                                    mybir.ActivationFunctionType.Copy,
                                    scale=1.0 / DIV,
                                )
                        if w0 == 0:
                            # JIT prefetch emitted mid-sc
                            load_until(h0 + 3 * sc_rows + first_chunk)

                    # edge fixup: recompute output columns 0 and 255
                    et = epool.tile(
                        [M, 2, sc_rows], mybir.dt.float32, name="et", tag="et"
                    )
                    ecount = [0, 0]
                    for e, entry in enumerate(EDGE_PAIRS):
                        grp = entry[2]
                        s = N_BULK + e
                        nc.tensor.matmul(
                            et[:, grp, :],
                            wsb[:, s, :, :],
                            rhs_edge(h0, sc_rows, entry),
                            start=(ecount[grp] == 0),
                            stop=(ecount[grp] == 3),
                            perf_mode=mybir.MatmulPerfMode.DoubleRow,
                        )
                        ecount[grp] += 1
                    for grp, c in ((0, 0), (1, W - 1)):
                        nc.vector.tensor_scalar_mul(
                            ot[:, :, c : c + 1],
                            et[:, grp, :].unsqueeze(2),
                            1.0 / DIV,
                        )
                    nc.sync.dma_start(
                        out=out_dram[:, h0 : h0 + sc_rows, :], in_=ot[:]
                    )
                load_until(H)

            for _rep in range(repeats):
                emit_body()

    nc.compile()
    return nc


def make_weight_tensor(weight):
    """Binarize + block-diagonalize into the paired slot layout."""
    wbin = np.where(weight > 0, 1.0, -1.0).astype(np.float32)
    # [co, ci, kh, kw] -> per-tap [ci, co]
    wt = wbin.reshape(C_OUT, C_IN, K * K).transpose(1, 2, 0)
    w128 = np.zeros((K * K, P, M), dtype=np.float32)
    for i in range(N_IMG):
        w128[:, i * C_IN : (i + 1) * C_IN, i * C_OUT : (i + 1) * C_OUT] = (
            wt.transpose(1, 0, 2)
        )
    wall = np.zeros((P, N_SLOTS, 2, M), dtype=np.float32)
    for s, pair in enumerate(BULK_PAIRS):
        for pl, (pk, dy, dx) in enumerate(pair):
            wall[:, s, pl, :] = w128[dy * K + dx]
    for e, entry in enumerate(EDGE_PAIRS):
        for pl, (pk, dy, dx) in enumerate(entry[:2]):
            if pk is not None:
                wall[:, N_BULK + e, pl, :] = w128[dy * K + dx]
    return wall.astype(ml_dtypes.float8_e4m3)


def kernel(x, weight, trace=False, repeats=1, _nc_cache={}):
    """Full-input entry point: x (32,32,256,256) f32, weight (32,32,3,3) f32."""
    x = np.asarray(x, dtype=np.float32)
    weight = np.asarray(weight, dtype=np.float32)
    n_batch = x.shape[0]
    per_core = n_batch // N_CORES

    if repeats not in _nc_cache:
        _nc_cache[repeats] = build_conv_kernel(repeats=repeats)
    nc = _nc_cache[repeats]

    wall = make_weight_tensor(weight)
    # dual-plane e4m3 host marshalling: a = fp8(x), b = fp8(x - a),
    # row-interleaved: [n, cin, h, 2, w]
    a = x.astype(ml_dtypes.float8_e4m3)
    b = (x - a.astype(np.float32)).astype(ml_dtypes.float8_e4m3)
    xab = np.stack([a, b], axis=3)

    in_maps = [
        {
            "xab": np.ascontiguousarray(
                xab[i * per_core : (i + 1) * per_core].reshape(P, H, 2, W)
            ),
            "wall": wall,
        }
        for i in range(N_CORES)
    ]
    try:
        res = bass_utils.run_bass_kernel_spmd(
            nc, in_maps, core_ids=list(range(N_CORES)), trace=trace
        )
    except ModuleNotFoundError:
        res = bass_utils.run_bass_kernel_spmd(
            nc, in_maps, core_ids=list(range(N_CORES)), trace=False
        )
    out = np.concatenate(
        [r["out"].astype(np.float32).reshape(per_core, C_OUT, H, W)
         for r in res.results],
        axis=0,
    )
    if trace:
        kernel.last_results = res
    return out


# revision 34
# speedup vs baseline: 2.3171x; 1.0196x over previous
"""Binarized 3x3 conv (BConv2d) on 8 TRN2 NeuronCores — fp8 DoubleRow version.

Problem: x (32, 32, 256, 256) f32, weight (32, 32, 3, 3) f32.
  out = conv2d(x, sign(weight), padding='same') / sqrt(32*9)

Strategy:
  - Data-parallel over batch: core i gets images 4i..4i+3 (no collectives).
  - Block-diagonal image packing: K = M = 128 = 4 images x 32 channels.
  - fp8 e4m3 DoubleRow matmuls: one instruction computes
    W_A.T @ X_A + W_B.T @ X_B at 0.5 cycles/column (4x bf16 rate).
    The A/B planes are expressed as a [K, 2, N] rhs access pattern whose
    middle dim strides between two tap windows of the same SBUF image
    (hardware requires that stride to be EVEN; odd strides crash).
  - Dual-plane numerics: host ships a = e4m3(x) and b = e4m3(x - a).
    7 of the 9 taps also accumulate the b (residual) plane, recovering
    most of the e4m3 quantization error: rel err ~1.3e-2 vs 2e-2 gate
    (full e4m3 alone is 2.7e-2).  16 tap-planes = 8 DoubleRow per output
    row vs 9 fp16 matmuls per row in the fp16 design (3.6x fewer PE
    cycles).
  - Input layout is W-unpadded (rows contiguous) so fp8 chunks DMA at
    full descriptor speed; column-edge outputs (c=0, c=255) read
    wrapped garbage and are recomputed by small per-superchunk edge
    matmuls (N = rows), then overwritten during the drain.
  - fp32 PSUM accumulate, drains split across DVE and ACT engines with
    the 1/sqrt(288) scale fused, fp16 output DMA.
"""

import numpy as np
import ml_dtypes

import concourse.mybir as mybir
import concourse.tile as tile
from concourse import bacc
from concourse import bass_utils
from concourse.ap import AP

N_CORES = 8
N_IMG = 4
C_IN = 32
C_OUT = 32
K = 3
H = 256
W = 256
DIV = float(np.sqrt(C_IN * K * K))

P = N_IMG * C_IN      # 128 partitions (contraction)
M = N_IMG * C_OUT     # 128 psum partitions (outputs)
HP = H + 2            # zero row above and below
RS = 2 * W            # row stride: a-row and b-row interleaved per image row
                      # (keeps every DoubleRow pair delta < 2^16: the ISA
                      # step_elem field is 16-bit, a flat-plane delta of
                      # 66048 fails codegen)
GUARD = 8             # guard bytes (tap windows spill 1 elem at the edges)
XF = 2 * GUARD + HP * RS
A0 = GUARD            # a sub-row base offset within a row slot
B0 = GUARD + W        # b sub-row base offset within a row slot

# ---- bulk tap-plane pairing (one DoubleRow instruction per entry) ----
# ('a'|'b'|None, dy, dx): None = zero-weight plane (reads in-bounds bytes).
# Constraint: within a pair, the flat-offset delta must be even, i.e. the
# dx values must have equal parity.  B-set (residual-corrected taps) =
# {(1,1),(0,0),(1,0),(2,0),(1,2)}; uncorrected: (0,1),(2,1),(0,2),(2,2)
# (rel err ~1.77e-2 vs the 2e-2 gate; 7 instructions/row).
BULK_PAIRS = [
    (("a", 0, 0), ("a", 0, 2)),
    (("a", 1, 0), ("a", 1, 2)),
    (("a", 2, 0), ("a", 2, 2)),
    (("a", 0, 1), ("a", 1, 1)),
    (("b", 1, 1), ("a", 2, 1)),
    (("b", 0, 0), ("b", 1, 0)),
    (("b", 1, 2), ("b", 2, 0)),
]
# ---- edge fixup pairs (a-plane only; error contribution ~2/256 pixels) --
# col 0 rebuilt from taps dx in {1,2} (input cols 0,1); col 255 from taps
# dx in {0,1} (input cols 254,255).  EDGE_COL maps slot -> (psum row, cin0)
EDGE_PAIRS = [
    # column 0 (group 0): window offset = dy*W + (dx-1) + 0
    (("a", 0, 1), ("a", 1, 1), 0, 0),
    (("a", 2, 1), (None, 2, 1), 0, 0),
    (("a", 0, 2), ("a", 1, 2), 0, 0),
    (("a", 2, 2), (None, 2, 2), 0, 0),
    # column 255 (group 1): window offset = dy*W + (dx-1) + 255
    (("a", 0, 0), ("a", 1, 0), 1, 255),
    (("a", 2, 0), (None, 2, 0), 1, 255),
    (("a", 0, 1), ("a", 1, 1), 1, 255),
    (("a", 2, 1), (None, 2, 1), 1, 255),
]
N_BULK = len(BULK_PAIRS)            # 8
N_EDGE = len(EDGE_PAIRS)            # 8
N_SLOTS = N_BULK + N_EDGE + 1       # +1 all-zero warmup slot
WARM_SLOT = N_SLOTS - 1


def _plane_off(plane, dy, dx):
    """Flat offset of tap window start relative to output row r's base
    (r*RS), for output column 0."""
    base = A0 if plane == "a" else B0
    return base + dy * RS + (dx - 1)


def build_conv_kernel(
    sc_rows=16,        # output rows per superchunk (drain+DMA granularity)
    chunk_rows=4,      # input DMA chunk rows (both planes, contiguous)
    first_chunk=-16,   # upfront load = 2*sc_rows+this rows (16: sc0 half + halo)
    warmup_mms=24,     # zero-weight DoubleRow matmuls to ramp the PE clock
    psum_bufs=6,       # bulk PSUM rotation depth
    obufs=4,           # output staging buffers
    in_queue="act",    # engine queue for input DMAs ('act' | 'sp')
    wave=4,            # pairs per staggered wave (psum banks held)
    drain_split=True,  # alternate drains DVE/ACT (False: all DVE)
    repeats=1,
):
    assert H % sc_rows == 0 and sc_rows % 2 == 0
    n_sc = H // sc_rows

    nc = bacc.Bacc(
        "TRN2", target_bir_lowering=False, debug=False, num_devices=N_CORES
    )
    x_dram = nc.dram_tensor(
        "xab", [P, H, 2, W], mybir.dt.float8e4, kind="ExternalInput"
    )
    w_dram = nc.dram_tensor(
        "wall", [P, N_SLOTS, 2, M], mybir.dt.float8e4, kind="ExternalInput"
    )
    out_dram = nc.dram_tensor(
        "out", [M, H, W], mybir.dt.float16, kind="ExternalOutput"
    )

    with tile.TileContext(nc) as tc:
        with (
            tc.tile_pool(name="persist", bufs=1) as perpool,
            tc.tile_pool(name="ostage", bufs=obufs) as opool,
            tc.tile_pool(name="psum", bufs=psum_bufs, space="PSUM") as ppool,
            tc.tile_pool(name="epsum", bufs=2, space="PSUM") as epool,
        ):
            xab = perpool.tile([P, XF], mybir.dt.float8e4, name="xab")
            wsb = perpool.tile(
                [P, N_SLOTS, 2, M], mybir.dt.float8e4, name="wsb"
            )
            nc.sync.dma_start(out=wsb[:], in_=w_dram[:])

            # zero the guards and the top/bottom pad row slots (both the
            # a and b sub-rows of padded rows 0 and 257)
            nc.vector.memset(xab[:, 0:A0], 0.0)
            nc.vector.memset(xab[:, A0 : A0 + RS], 0.0)
            nc.vector.memset(xab[:, A0 + (HP - 1) * RS : XF], 0.0)

            xt = xab[:].tensor

            def rhs_bulk(r, pair):
                (p1, dy1, dx1), (p2, dy2, dx2) = pair
                o1 = r * RS + _plane_off(p1, dy1, dx1)
                o2 = r * RS + _plane_off(p2, dy2, dx2)
                assert (o2 - o1) % 2 == 0 and o1 >= 0 and 0 < o2 - o1 < 65536
                return AP(xt, o1, [[XF, P], [o2 - o1, 2], [1, W]])

            def rhs_edge(r0, rows, entry):
                (p1, dy1, dx1), (p2, dy2, dx2), _grp, c = entry
                o1 = r0 * RS + _plane_off(p1, dy1, dx1) + c
                if p2 is None:
                    o2 = o1 + 2   # zero-weight plane: any even in-bounds delta
                else:
                    o2 = r0 * RS + _plane_off(p2, dy2, dx2) + c
                assert (o2 - o1) % 2 == 0 and o1 >= 0 and 0 < o2 - o1 < 65536
                return AP(xt, o1, [[XF, P], [o2 - o1, 2], [RS, rows]])

            def emit_input_rows(r0, nrows):
                # the row-interleaved [P, H, 2, W] dram layout makes each
                # chunk one fully contiguous transfer (both planes)
                eng = {"act": nc.scalar, "sp": nc.sync,
                       "pool": nc.gpsimd}[in_queue]
                dst = AP(
                    xt, A0 + (r0 + 1) * RS, [[XF, P], [1, nrows * RS]]
                )
                eng.dma_start(
                    out=dst,
                    in_=x_dram[:, r0 : r0 + nrows, :, :],
                )

            def emit_body():
                next_row = [0]

                def load_until(row_needed):
                    while next_row[0] < min(row_needed, H):
                        r0 = next_row[0]
                        nrows = min(chunk_rows, H - r0)
                        emit_input_rows(r0, nrows)
                        next_row[0] += nrows

                # PE p-state warm-up on the (memset) guard bytes.  Emitted
                # BEFORE the upfront loads: any xab read emitted after a DMA
                # inherits a wait on it (latest-emitted coalescing), which
                # would make the warmups wait out the whole upfront burst.
                if warmup_mms:
                    wpt = ppool.tile(
                        [M, 2, W], mybir.dt.float32, name="pt", tag="pt"
                    )
                    # memset zero-weight tile: warmups depend on no DMA, so
                    # the p-state ramp starts while weights are in flight
                    wz = perpool.tile([P, 2, M], mybir.dt.float8e4, name="wz")
                    nc.vector.memset(wz[:], 0.0)
                    # read only the (memset) a-plane top pad row: base A0,
                    # plane delta 2, N=254 keeps every byte inside the pad
                    wrhs = AP(xt, A0, [[XF, P], [2, 2], [1, W - 2]])
                    for _ in range(warmup_mms):
                        nc.tensor.matmul(
                            wpt[:, 0, 0 : W - 2],
                            wz[:],
                            wrhs,
                            start=True,
                            stop=True,
                            perf_mode=mybir.MatmulPerfMode.DoubleRow,
                        )

                # upfront: superchunk 0, its halo, and one chunk of slack
                load_until(2 * sc_rows + first_chunk)

                for sc in range(n_sc):
                    h0 = sc * sc_rows
                    ot = opool.tile(
                        [M, sc_rows, W], mybir.dt.float16, name="ot", tag="ot"
                    )
                    n_pairs = sc_rows // 2
                    w_step = 2 if last else wave
                    for w0 in range(0, n_pairs, w_step):
                        pts = [
                            ppool.tile(
                                [M, 2, W], mybir.dt.float32,
                                name="pt", tag="pt",
                            )
                            for _ in range(min(w_step, n_pairs - w0))
                        ]
                        # phase 1: a-plane slots for the whole wave; phase 2:
                        # b-plane slots.  This defers the wave's first
                        # b-chunk semaphore wait by ~2 us of PE work, past
                        # the prefetched chunk's landing time.
                        for s, pair in enumerate(BULK_PAIRS):
                            for jj, pt in enumerate(pts):
                                r0 = h0 + 2 * (w0 + jj)
                                for i in range(2):
                                    nc.tensor.matmul(
                                        pt[:, i, :],
                                        wsb[:, s, :, :],
                                        rhs_bulk(r0 + i, pair),
                                        start=(s == 0),
                                        stop=(s == N_BULK - 1),
                                        perf_mode=mybir.MatmulPerfMode.DoubleRow,
                                    )
                        for jj, pt in enumerate(pts):
                            j = w0 + jj
                            dst = ot[:, 2 * j : 2 * j + 2, :]
                            if j % 2 == 0 or not drain_split:
                                nc.vector.tensor_scalar_mul(
                                    dst, pt[:], 1.0 / DIV
                                )
                            else:
                                nc.scalar.activation(
                                    dst,
                                    pt[:],
                                    mybir.ActivationFunctionType.Copy,
                                    scale=1.0 / DIV,
                                )
                        if w0 == 0:
                            # JIT prefetch emitted mid-sc
                            load_until(h0 + 3 * sc_rows + first_chunk)

                    # edge fixup: recompute output columns 0 and 255
                    et = epool.tile(
                        [M, 2, sc_rows], mybir.dt.float32, name="et", tag="et"
                    )
                    ecount = [0, 0]
                    for e, entry in enumerate(EDGE_PAIRS):
                        grp = entry[2]
                        s = N_BULK + e
                        nc.tensor.matmul(
                            et[:, grp, :],
                            wsb[:, s, :, :],
                            rhs_edge(h0, sc_rows, entry),
                            start=(ecount[grp] == 0),
                            stop=(ecount[grp] == 3),
                            perf_mode=mybir.MatmulPerfMode.DoubleRow,
                        )
                        ecount[grp] += 1
                    for grp, c in ((0, 0), (1, W - 1)):
                        nc.vector.tensor_scalar_mul(
                            ot[:, :, c : c + 1],
                            et[:, grp, :].unsqueeze(2),
                            1.0 / DIV,
                        )
                    nc.sync.dma_start(
                        out=out_dram[:, h0 : h0 + sc_rows, :], in_=ot[:]
                    )
                load_until(H)

            for _rep in range(repeats):
                emit_body()

    nc.compile()
    return nc


def make_weight_tensor(weight):
    """Binarize + block-diagonalize into the paired slot layout."""
    wbin = np.where(weight > 0, 1.0, -1.0).astype(np.float32)
    # [co, ci, kh, kw] -> per-tap [ci, co]
    wt = wbin.reshape(C_OUT, C_IN, K * K).transpose(1, 2, 0)
    w128 = np.zeros((K * K, P, M), dtype=np.float32)
    for i in range(N_IMG):
        w128[:, i * C_IN : (i + 1) * C_IN, i * C_OUT : (i + 1) * C_OUT] = (
            wt.transpose(1, 0, 2)
        )
    wall = np.zeros((P, N_SLOTS, 2, M), dtype=np.float32)
    for s, pair in enumerate(BULK_PAIRS):
        for pl, (pk, dy, dx) in enumerate(pair):
            wall[:, s, pl, :] = w128[dy * K + dx]
    for e, entry in enumerate(EDGE_PAIRS):
        for pl, (pk, dy, dx) in enumerate(entry[:2]):
            if pk is not None:
                wall[:, N_BULK + e, pl, :] = w128[dy * K + dx]
    return wall.astype(ml_dtypes.float8_e4m3)


def kernel(x, weight, trace=False, repeats=1, _nc_cache={}):
    """Full-input entry point: x (32,32,256,256) f32, weight (32,32,3,3) f32."""
    x = np.asarray(x, dtype=np.float32)
    weight = np.asarray(weight, dtype=np.float32)
    n_batch = x.shape[0]
    per_core = n_batch // N_CORES

    if repeats not in _nc_cache:
        _nc_cache[repeats] = build_conv_kernel(repeats=repeats)
    nc = _nc_cache[repeats]

    wall = make_weight_tensor(weight)
    # dual-plane e4m3 host marshalling: a = fp8(x), b = fp8(x - a),
    # row-interleaved: [n, cin, h, 2, w]
    a = x.astype(ml_dtypes.float8_e4m3)
    b = (x - a.astype(np.float32)).astype(ml_dtypes.float8_e4m3)
    xab = np.stack([a, b], axis=3)

    in_maps = [
        {
            "xab": np.ascontiguousarray(
                xab[i * per_core : (i + 1) * per_core].reshape(P, H, 2, W)
            ),
            "wall": wall,
        }
        for i in range(N_CORES)
    ]
    try:
        res = bass_utils.run_bass_kernel_spmd(
            nc, in_maps, core_ids=list(range(N_CORES)), trace=trace
        )
    except ModuleNotFoundError:
        res = bass_utils.run_bass_kernel_spmd(
            nc, in_maps, core_ids=list(range(N_CORES)), trace=False
        )
    out = np.concatenate(
        [r["out"].astype(np.float32).reshape(per_core, C_OUT, H, W)
         for r in res.results],
        axis=0,
    )
    if trace:
        kernel.last_results = res
    return out


# revision 35
# speedup vs baseline: 2.3190x; 1.0008x over previous
"""Binarized 3x3 conv (BConv2d) on 8 TRN2 NeuronCores — fp8 DoubleRow version.

Problem: x (32, 32, 256, 256) f32, weight (32, 32, 3, 3) f32.
  out = conv2d(x, sign(weight), padding='same') / sqrt(32*9)

Strategy:
  - Data-parallel over batch: core i gets images 4i..4i+3 (no collectives).
  - Block-diagonal image packing: K = M = 128 = 4 images x 32 channels.
  - fp8 e4m3 DoubleRow matmuls: one instruction computes
    W_A.T @ X_A + W_B.T @ X_B at 0.5 cycles/column (4x bf16 rate).
    The A/B planes are expressed as a [K, 2, N] rhs access pattern whose
    middle dim strides between two tap windows of the same SBUF image
    (hardware requires that stride to be EVEN; odd strides crash).
  - Dual-plane numerics: host ships a = e4m3(x) and b = e4m3(x - a).
    7 of the 9 taps also accumulate the b (residual) plane, recovering
    most of the e4m3 quantization error: rel err ~1.3e-2 vs 2e-2 gate
    (full e4m3 alone is 2.7e-2).  16 tap-planes = 8 DoubleRow per output
    row vs 9 fp16 matmuls per row in the fp16 design (3.6x fewer PE
    cycles).
  - Input layout is W-unpadded (rows contiguous) so fp8 chunks DMA at
    full descriptor speed; column-edge outputs (c=0, c=255) read
    wrapped garbage and are recomputed by small per-superchunk edge
    matmuls (N = rows), then overwritten during the drain.
  - fp32 PSUM accumulate, drains split across DVE and ACT engines with
    the 1/sqrt(288) scale fused, fp16 output DMA.
"""

import numpy as np
import ml_dtypes

import concourse.mybir as mybir
import concourse.tile as tile
from concourse import bacc
from concourse import bass_utils
from concourse.ap import AP

N_CORES = 8
N_IMG = 4
C_IN = 32
C_OUT = 32
K = 3
H = 256
W = 256
DIV = float(np.sqrt(C_IN * K * K))

P = N_IMG * C_IN      # 128 partitions (contraction)
M = N_IMG * C_OUT     # 128 psum partitions (outputs)
HP = H + 2            # zero row above and below
RS = 2 * W            # row stride: a-row and b-row interleaved per image row
                      # (keeps every DoubleRow pair delta < 2^16: the ISA
                      # step_elem field is 16-bit, a flat-plane delta of
                      # 66048 fails codegen)
GUARD = 8             # guard bytes (tap windows spill 1 elem at the edges)
XF = 2 * GUARD + HP * RS
A0 = GUARD            # a sub-row base offset within a row slot
B0 = GUARD + W        # b sub-row base offset within a row slot

# ---- bulk tap-plane pairing (one DoubleRow instruction per entry) ----
# ('a'|'b'|None, dy, dx): None = zero-weight plane (reads in-bounds bytes).
# Constraint: within a pair, the flat-offset delta must be even, i.e. the
# dx values must have equal parity.  B-set (residual-corrected taps) =
# {(1,1),(0,0),(1,0),(2,0),(1,2)}; uncorrected: (0,1),(2,1),(0,2),(2,2)
# (rel err ~1.77e-2 vs the 2e-2 gate; 7 instructions/row).
BULK_PAIRS = [
    (("a", 0, 0), ("a", 0, 2)),
    (("a", 1, 0), ("a", 1, 2)),
    (("a", 2, 0), ("a", 2, 2)),
    (("a", 0, 1), ("a", 1, 1)),
    (("b", 1, 1), ("a", 2, 1)),
    (("b", 0, 0), ("b", 1, 0)),
    (("b", 1, 2), ("b", 2, 0)),
]
# ---- edge fixup pairs (a-plane only; error contribution ~2/256 pixels) --
# col 0 rebuilt from taps dx in {1,2} (input cols 0,1); col 255 from taps
# dx in {0,1} (input cols 254,255).  EDGE_COL maps slot -> (psum row, cin0)
EDGE_PAIRS = [
    # column 0 (group 0): window offset = dy*W + (dx-1) + 0
    (("a", 0, 1), ("a", 1, 1), 0, 0),
    (("a", 2, 1), (None, 2, 1), 0, 0),
    (("a", 0, 2), ("a", 1, 2), 0, 0),
    (("a", 2, 2), (None, 2, 2), 0, 0),
    # column 255 (group 1): window offset = dy*W + (dx-1) + 255
    (("a", 0, 0), ("a", 1, 0), 1, 255),
    (("a", 2, 0), (None, 2, 0), 1, 255),
    (("a", 0, 1), ("a", 1, 1), 1, 255),
    (("a", 2, 1), (None, 2, 1), 1, 255),
]
N_BULK = len(BULK_PAIRS)            # 8
N_EDGE = len(EDGE_PAIRS)            # 8
N_SLOTS = N_BULK + N_EDGE + 1       # +1 all-zero warmup slot
WARM_SLOT = N_SLOTS - 1


def _plane_off(plane, dy, dx):
    """Flat offset of tap window start relative to output row r's base
    (r*RS), for output column 0."""
    base = A0 if plane == "a" else B0
    return base + dy * RS + (dx - 1)


def build_conv_kernel(
    sc_rows=16,        # output rows per superchunk (drain+DMA granularity)
    chunk_rows=4,      # input DMA chunk rows (both planes, contiguous)
    first_chunk=-16,   # upfront load = 2*sc_rows+this rows (16: sc0 half + halo)
    warmup_mms=16,     # zero-weight DoubleRow matmuls to ramp the PE clock
    psum_bufs=6,       # bulk PSUM rotation depth
    obufs=4,           # output staging buffers
    in_queue="act",    # engine queue for input DMAs ('act' | 'sp')
    wave=4,            # pairs per staggered wave (psum banks held)
    drain_split=True,  # alternate drains DVE/ACT (False: all DVE)
    repeats=1,
):
    assert H % sc_rows == 0 and sc_rows % 2 == 0
    n_sc = H // sc_rows

    nc = bacc.Bacc(
        "TRN2", target_bir_lowering=False, debug=False, num_devices=N_CORES
    )
    x_dram = nc.dram_tensor(
        "xab", [P, H, 2, W], mybir.dt.float8e4, kind="ExternalInput"
    )
    w_dram = nc.dram_tensor(
        "wall", [P, N_SLOTS, 2, M], mybir.dt.float8e4, kind="ExternalInput"
    )
    out_dram = nc.dram_tensor(
        "out", [M, H, W], mybir.dt.float16, kind="ExternalOutput"
    )

    with tile.TileContext(nc) as tc:
        with (
            tc.tile_pool(name="persist", bufs=1) as perpool,
            tc.tile_pool(name="ostage", bufs=obufs) as opool,
            tc.tile_pool(name="psum", bufs=psum_bufs, space="PSUM") as ppool,
            tc.tile_pool(name="epsum", bufs=2, space="PSUM") as epool,
        ):
            xab = perpool.tile([P, XF], mybir.dt.float8e4, name="xab")
            wsb = perpool.tile(
                [P, N_SLOTS, 2, M], mybir.dt.float8e4, name="wsb"
            )
            nc.sync.dma_start(out=wsb[:], in_=w_dram[:])

            # zero the guards and the top/bottom pad row slots (both the
            # a and b sub-rows of padded rows 0 and 257)
            nc.vector.memset(xab[:, 0:A0], 0.0)
            nc.vector.memset(xab[:, A0 : A0 + RS], 0.0)
            nc.vector.memset(xab[:, A0 + (HP - 1) * RS : XF], 0.0)

            xt = xab[:].tensor

            def rhs_bulk(r, pair):
                (p1, dy1, dx1), (p2, dy2, dx2) = pair
                o1 = r * RS + _plane_off(p1, dy1, dx1)
                o2 = r * RS + _plane_off(p2, dy2, dx2)
                assert (o2 - o1) % 2 == 0 and o1 >= 0 and 0 < o2 - o1 < 65536
                return AP(xt, o1, [[XF, P], [o2 - o1, 2], [1, W]])

            def rhs_edge(r0, rows, entry):
                (p1, dy1, dx1), (p2, dy2, dx2), _grp, c = entry
                o1 = r0 * RS + _plane_off(p1, dy1, dx1) + c
                if p2 is None:
                    o2 = o1 + 2   # zero-weight plane: any even in-bounds delta
                else:
                    o2 = r0 * RS + _plane_off(p2, dy2, dx2) + c
                assert (o2 - o1) % 2 == 0 and o1 >= 0 and 0 < o2 - o1 < 65536
                return AP(xt, o1, [[XF, P], [o2 - o1, 2], [RS, rows]])

            def emit_input_rows(r0, nrows):
                # the row-interleaved [P, H, 2, W] dram layout makes each
                # chunk one fully contiguous transfer (both planes)
                eng = {"act": nc.scalar, "sp": nc.sync,
                       "pool": nc.gpsimd}[in_queue]
                dst = AP(
                    xt, A0 + (r0 + 1) * RS, [[XF, P], [1, nrows * RS]]
                )
                eng.dma_start(
                    out=dst,
                    in_=x_dram[:, r0 : r0 + nrows, :, :],
                )

            def emit_body():
                next_row = [0]

                def load_until(row_needed):
                    while next_row[0] < min(row_needed, H):
                        r0 = next_row[0]
                        nrows = min(chunk_rows, H - r0)
                        emit_input_rows(r0, nrows)
                        next_row[0] += nrows

                # PE p-state warm-up on the (memset) guard bytes.  Emitted
                # BEFORE the upfront loads: any xab read emitted after a DMA
                # inherits a wait on it (latest-emitted coalescing), which
                # would make the warmups wait out the whole upfront burst.
                if warmup_mms:
                    wpt = ppool.tile(
                        [M, 2, W], mybir.dt.float32, name="pt", tag="pt"
                    )
                    # memset zero-weight tile: warmups depend on no DMA, so
                    # the p-state ramp starts while weights are in flight
                    wz = perpool.tile([P, 2, M], mybir.dt.float8e4, name="wz")
                    nc.vector.memset(wz[:], 0.0)
                    # read only the (memset) a-plane top pad row: base A0,
                    # plane delta 2, N=254 keeps every byte inside the pad
                    wrhs = AP(xt, A0, [[XF, P], [2, 2], [1, W - 2]])
                    for _ in range(warmup_mms):
                        nc.tensor.matmul(
                            wpt[:, 0, 0 : W - 2],
                            wz[:],
                            wrhs,
                            start=True,
                            stop=True,
                            perf_mode=mybir.MatmulPerfMode.DoubleRow,
                        )

                # upfront: superchunk 0, its halo, and one chunk of slack
                load_until(2 * sc_rows + first_chunk)

                for sc in range(n_sc):
                    h0 = sc * sc_rows
                    ot = opool.tile(
                        [M, sc_rows, W], mybir.dt.float16, name="ot", tag="ot"
                    )
                    n_pairs = sc_rows // 2
                    w_step = 2 if last else wave
                    for w0 in range(0, n_pairs, w_step):
                        pts = [
                            ppool.tile(
                                [M, 2, W], mybir.dt.float32,
                                name="pt", tag="pt",
                            )
                            for _ in range(min(w_step, n_pairs - w0))
                        ]
                        # phase 1: a-plane slots for the whole wave; phase 2:
                        # b-plane slots.  This defers the wave's first
                        # b-chunk semaphore wait by ~2 us of PE work, past
                        # the prefetched chunk's landing time.
                        for s, pair in enumerate(BULK_PAIRS):
                            for jj, pt in enumerate(pts):
                                r0 = h0 + 2 * (w0 + jj)
                                for i in range(2):
                                    nc.tensor.matmul(
                                        pt[:, i, :],
                                        wsb[:, s, :, :],
                                        rhs_bulk(r0 + i, pair),
                                        start=(s == 0),
                                        stop=(s == N_BULK - 1),
                                        perf_mode=mybir.MatmulPerfMode.DoubleRow,
                                    )
                        for jj, pt in enumerate(pts):
                            j = w0 + jj
                            dst = ot[:, 2 * j : 2 * j + 2, :]
                            if j % 2 == 0 or not drain_split:
                                nc.vector.tensor_scalar_mul(
                                    dst, pt[:], 1.0 / DIV
                                )
                            else:
                                nc.scalar.activation(
                                    dst,
                                    pt[:],
                                    mybir.ActivationFunctionType.Copy,
                                    scale=1.0 / DIV,
                                )
                        if w0 == 0:
                            # JIT prefetch emitted mid-sc
                            load_until(h0 + 3 * sc_rows + first_chunk)

                    # edge fixup: recompute output columns 0 and 255
                    et = epool.tile(
                        [M, 2, sc_rows], mybir.dt.float32, name="et", tag="et"
                    )
                    ecount = [0, 0]
                    for e, entry in enumerate(EDGE_PAIRS):
                        grp = entry[2]
                        s = N_BULK + e
                        nc.tensor.matmul(
                            et[:, grp, :],
                            wsb[:, s, :, :],
                            rhs_edge(h0, sc_rows, entry),
                            start=(ecount[grp] == 0),
                            stop=(ecount[grp] == 3),
                            perf_mode=mybir.MatmulPerfMode.DoubleRow,
                        )
                        ecount[grp] += 1
                    for grp, c in ((0, 0), (1, W - 1)):
                        nc.vector.tensor_scalar_mul(
                            ot[:, :, c : c + 1],
                            et[:, grp, :].unsqueeze(2),
                            1.0 / DIV,
                        )
                    nc.sync.dma_start(
                        out=out_dram[:, h0 : h0 + sc_rows, :], in_=ot[:]
                    )
                load_until(H)

            for _rep in range(repeats):
                emit_body()

    nc.compile()
    return nc


def make_weight_tensor(weight):
    """Binarize + block-diagonalize into the paired slot layout."""
    wbin = np.where(weight > 0, 1.0, -1.0).astype(np.float32)
    # [co, ci, kh, kw] -> per-tap [ci, co]
    wt = wbin.reshape(C_OUT, C_IN, K * K).transpose(1, 2, 0)
    w128 = np.zeros((K * K, P, M), dtype=np.float32)
    for i in range(N_IMG):
        w128[:, i * C_IN : (i + 1) * C_IN, i * C_OUT : (i + 1) * C_OUT] = (
            wt.transpose(1, 0, 2)
        )
    wall = np.zeros((P, N_SLOTS, 2, M), dtype=np.float32)
    for s, pair in enumerate(BULK_PAIRS):
        for pl, (pk, dy, dx) in enumerate(pair):
            wall[:, s, pl, :] = w128[dy * K + dx]
    for e, entry in enumerate(EDGE_PAIRS):
        for pl, (pk, dy, dx) in enumerate(entry[:2]):
            if pk is not None:
                wall[:, N_BULK + e, pl, :] = w128[dy * K + dx]
    return wall.astype(ml_dtypes.float8_e4m3)


def kernel(x, weight, trace=False, repeats=1, _nc_cache={}):
    """Full-input entry point: x (32,32,256,256) f32, weight (32,32,3,3) f32."""
    x = np.asarray(x, dtype=np.float32)
    weight = np.asarray(weight, dtype=np.float32)
    n_batch = x.shape[0]
    per_core = n_batch // N_CORES

    if repeats not in _nc_cache:
        _nc_cache[repeats] = build_conv_kernel(repeats=repeats)
    nc = _nc_cache[repeats]

    wall = make_weight_tensor(weight)
    # dual-plane e4m3 host marshalling: a = fp8(x), b = fp8(x - a),
    # row-interleaved: [n, cin, h, 2, w]
    a = x.astype(ml_dtypes.float8_e4m3)
    b = (x - a.astype(np.float32)).astype(ml_dtypes.float8_e4m3)
    xab = np.stack([a, b], axis=3)

    in_maps = [
        {
            "xab": np.ascontiguousarray(
                xab[i * per_core : (i + 1) * per_core].reshape(P, H, 2, W)
            ),
            "wall": wall,
        }
        for i in range(N_CORES)
    ]
    try:
        res = bass_utils.run_bass_kernel_spmd(
            nc, in_maps, core_ids=list(range(N_CORES)), trace=trace
        )
    except ModuleNotFoundError:
        res = bass_utils.run_bass_kernel_spmd(
            nc, in_maps, core_ids=list(range(N_CORES)), trace=False
        )
    out = np.concatenate(
        [r["out"].astype(np.float32).reshape(per_core, C_OUT, H, W)
         for r in res.results],
        axis=0,
    )
    if trace:
        kernel.last_results = res
    return out
